# revision 10
# baseline (speedup 1.0000x reference)
"""DNC single-step forward on 8 Trainium2 NeuronCores (Bass/Tile), v2.

Data-parallel over batch (B=256 -> 32/core); memory/link/usage/params
replicated. Cross-core collectives:
  - one AllGather: [allocation-weight shard | link row-sum shard]
  - one AllReduce: [erase/add matrix partials | link col-sum partial |
    masked-lu sum partial]

v2 layout/scheduling changes vs v1:
  - LSTM gates computed batch-stationary: psum (32, 2048)+(32, 1024),
    free=512 matmuls, bias folded in as a 13th contraction chunk whose
    lhsT column is e_0 and whose rhs row 0 is the bias.
  - all matmul operands bf16 (weights, normalized memory, write weights,
    read weights); link/memory DMA'd in bf16.
  - alloc shard + link stats first -> AllGather fires ~13us, hidden
    under the LSTM.
  - output projection h-part accumulates in psum during the AllReduce;
    only the read-out part remains on the post-collective path.
  - row norms via scalar-engine Square+accum; bf16 PE transposes.
"""
import sys

sys.path.insert(0, '/opt/trn_rl_repo')

import numpy as np
import ml_dtypes
import concourse.bass as bass
import concourse.bacc as bacc
import concourse.tile as tile
from concourse import mybir
from concourse.bass_utils import run_bass_kernel_spmd
from concourse.masks import make_identity

AF = mybir.ActivationFunctionType
ALU = mybir.AluOpType
F32 = mybir.dt.float32
BF16 = mybir.dt.bfloat16

B, N, D, R, H, I = 256, 2048, 128, 4, 1024, 1024
CI = I + R * D          # 1536
IF = 787
M = 8                   # cores
BL = B // M             # 32 batch rows per core
NT = N // 128           # 16 n-tiles
KC = CI // 128          # 12 k-chunks of cin
KH = H // 128           # 8 k-chunks of x/h
NSH = N // M            # 256 usage/alloc shard

ARA = NT * 128 * 256    # AR region A: [ep|ap] per n-tile
ARTOT = ARA + 2 * N     # + region B: [link colsum | lu stat]


def build_nc():
    nc = bacc.Bacc("TRN2", target_bir_lowering=False, num_devices=M)
    dt = F32
    cinx = nc.declare_dram_parameter("cinx", [128, KH * BL], BF16, isOutput=False)
    w3 = nc.declare_dram_parameter("w3", [128, KC * 3072], BF16, isOutput=False)
    b3r = nc.declare_dram_parameter("b3r", [1, 3072], BF16, isOutput=False)
    wif = nc.declare_dram_parameter("wif", [128, KH * IF], BF16, isOutput=False)
    bifr = nc.declare_dram_parameter("bifr", [1, IF], BF16, isOutput=False)
    wout = nc.declare_dram_parameter("wout", [128, KC * 1024], BF16, isOutput=False)
    borr = nc.declare_dram_parameter("borr", [1, 1024], BF16, isOutput=False)
    memA = nc.declare_dram_parameter("memA", [128, N], BF16, isOutput=False)
    lnk = nc.declare_dram_parameter("lnk", [128, 2 * N], BF16, isOutput=False)
    usg = nc.declare_dram_parameter("usg", [1, N], dt, isOutput=False)
    ucols = nc.declare_dram_parameter("ucols", [128, 2], dt, isOutput=False)
    outO = nc.declare_dram_parameter("outO", [BL, 1024], dt, isOutput=True)

    from contextlib import ExitStack
    with tile.TileContext(nc) as tc, ExitStack() as es:
        cons = es.enter_context(tc.tile_pool(name="cons", bufs=1))
        wk = es.enter_context(tc.tile_pool(name="wk", bufs=1))
        dpool = es.enter_context(tc.tile_pool(name="dram", bufs=1, space="DRAM"))
        ppq = es.enter_context(tc.tile_pool(name="ppq", bufs=1, space="PSUM"))
        ppf = es.enter_context(tc.tile_pool(name="ppf", bufs=1, space="PSUM"))
        ppt = es.enter_context(tc.tile_pool(name="ppt", bufs=2, space="PSUM"))

        ag_in = dpool.tile([2 * NSH], dt)
        ag_out = dpool.tile([M, 2 * NSH], dt, addr_space="Shared")
        ar_in = dpool.tile([ARTOT], BF16)
        ar_out = dpool.tile([ARTOT], BF16, addr_space="Shared")

        # ---- constants / identities ----
        ident = cons.tile([128, 128], dt)
        make_identity(nc, ident)
        identb = cons.tile([128, 128], BF16)
        make_identity(nc, identb)
        ones_colb = cons.tile([128, 1], BF16)
        nc.vector.memset(ones_colb, 1.0)
        ones32 = cons.tile([128, BL], dt)
        nc.vector.memset(ones32, 1.0)
        ones32b = cons.tile([BL, 1], BF16)
        nc.vector.memset(ones32b, 1.0)
        # bias-chunk lhsT: e0 column (row 0 = 1, rest 0)
        cin13 = cons.tile([128, BL], BF16)
        nc.vector.memset(cin13, 0.0)
        nc.vector.memset(cin13[0:1, :], 1.0)

        # ---- early input DMAs ----
        mem_sb = cons.tile([128, N], BF16)
        nc.sync.dma_start(out=mem_sb, in_=memA[:, :])
        usg_b = wk.tile([128, N], dt, tag="ubig")
        nc.sync.dma_start(out=usg_b, in_=usg[0:1, :].partition_broadcast(128))
        uc_sb = cons.tile([128, 2], dt)
        nc.sync.dma_start(out=uc_sb, in_=ucols[:, :])
        l0 = wk.tile([128, N], BF16, tag="l0")
        nc.sync.dma_start(out=l0[:, 0:1024], in_=lnk[:, 0:1024])
        nc.sync.dma_start(out=l0[:, 1024:2048], in_=lnk[:, 1024:2048])
        l1 = wk.tile([128, N], BF16, tag="l1")
        nc.sync.dma_start(out=l1[:, 0:1024], in_=lnk[:, N:N + 1024])
        nc.sync.dma_start(out=l1[:, 1024:2048], in_=lnk[:, N + 1024:2 * N])
        cin = wk.tile([128, KH, BL], BF16)
        nc.sync.dma_start(out=cin,
                          in_=cinx[:, :].rearrange("p (k b) -> p k b", b=BL))
        bif_b = wk.tile([BL, IF], BF16)
        nc.sync.dma_start(out=bif_b, in_=bifr[0:1, :].partition_broadcast(BL))

        # ============ Stage A: alloc shard + link stats -> AllGather ========
        # link rowsum shard
        for i, lt in enumerate((l0, l1)):
            rs = wk.tile([128, 1], dt, tag="rs", bufs=2)
            nc.vector.tensor_reduce(out=rs, in_=lt, axis=mybir.AxisListType.X,
                                    op=ALU.add)
            nc.sync.dma_start(out=ag_in[NSH + i * 128:NSH + (i + 1) * 128],
                              in_=rs)

        # alloc shard: a_i = u_i * exp(sum_{u_k<u_i} ln(1-u_k))
        L_b = wk.tile([128, N], dt, tag="lbig")
        nc.scalar.activation(out=L_b, in_=usg_b, func=AF.Ln, bias=1.0,
                             scale=-1.0)
        for i in range(2):
            u_col = uc_sb[:, i:i + 1]
            step = wk.tile([128, N], dt, tag="eaf", bufs=1)
            nc.vector.scalar_tensor_tensor(out=step, in0=usg_b, scalar=u_col,
                                           in1=L_b, op0=ALU.is_lt,
                                           op1=ALU.mult)
            a_col = wk.tile([128, 1], dt, tag="acol", bufs=2)
            nc.vector.tensor_reduce(out=a_col, in_=step,
                                    axis=mybir.AxisListType.X, op=ALU.add)
            nc.scalar.activation(out=a_col, in_=a_col, func=AF.Exp)
            nc.vector.tensor_mul(out=a_col, in0=a_col, in1=u_col)
            nc.sync.dma_start(out=ag_in[i * 128:(i + 1) * 128], in_=a_col)

        nc.gpsimd.collective_compute(
            "AllGather", ALU.bypass, replica_groups=[list(range(M))],
            ins=[ag_in[:]], outs=[ag_out.flatten()])

        # ---- weight streams (issued after the collective so its input-DMA
        # semaphore does not wait behind them) ----
        w3_sb = cons.tile([128, KC * 3072], BF16)
        for k in range(KC):
            hw = 1536
            nc.sync.dma_start(out=w3_sb[:, k * 3072:k * 3072 + hw],
                              in_=w3[:, k * 3072:k * 3072 + hw])
            nc.sync.dma_start(out=w3_sb[:, k * 3072 + hw:(k + 1) * 3072],
                              in_=w3[:, k * 3072 + hw:(k + 1) * 3072])
        wif_sb = wk.tile([128, KH * IF], BF16, tag="wif")
        for q in range(4):
            nc.sync.dma_start(out=wif_sb[:, q * 2 * IF:(q + 1) * 2 * IF],
                              in_=wif[:, q * 2 * IF:(q + 1) * 2 * IF])
        wout_sb = cons.tile([128, KC * 1024], BF16)
        for q in range(4):          # h-part, k=0..7
            nc.sync.dma_start(out=wout_sb[:, q * 2048:(q + 1) * 2048],
                              in_=wout[:, q * 2048:(q + 1) * 2048])
        for q in range(2):          # ro-part, k=8..11
            nc.sync.dma_start(out=wout_sb[:, 8192 + q * 2048:8192 + (q + 1) * 2048],
                              in_=wout[:, 8192 + q * 2048:8192 + (q + 1) * 2048])
        # bias rhs chunks (row 0 = bias, rest zero)
        w3b = cons.tile([128, 3072], BF16)
        nc.vector.memset(w3b, 0.0)
        nc.sync.dma_start(out=w3b[0:1, :], in_=b3r[:, :])
        woutb = cons.tile([128, 1024], BF16)
        nc.vector.memset(woutb, 0.0)
        nc.sync.dma_start(out=woutb[0:1, :], in_=borr[:, :])

        # mean column of memory (for the uniform-read controller input)
        mean_ps = ppf.tile([1, 128], dt, tag="f")
        for t in range(NT):
            nc.tensor.matmul(mean_ps, lhsT=ones_colb,
                             rhs=mem_sb[:, t * 128:(t + 1) * 128],
                             start=(t == 0), stop=(t == NT - 1))
        mean_row = wk.tile([1, 128], dt, tag="meanr")
        nc.scalar.activation(out=mean_row, in_=mean_ps, func=AF.Copy,
                             scale=1.0 / N)
        mc_ps = ppt.tile([128, 1], dt, tag="tr")
        nc.tensor.transpose(mc_ps, mean_row, ident[0:1, 0:1])
        mean_col = wk.tile([128, 1], dt, tag="meanc")
        nc.vector.tensor_copy(out=mean_col, in_=mc_ps)
        cmean = wk.tile([128, BL], BF16)
        nc.scalar.activation(out=cmean, in_=ones32, func=AF.Copy,
                             scale=mean_col)

        # link colsum partial (bf16 matmuls, 4 sequential 512-wide groups)
        cs_row = wk.tile([1, N], BF16, tag="csrow")
        for ch in range(4):
            cs_ps = ppf.tile([1, 512], dt, tag="f")
            nc.tensor.matmul(cs_ps, lhsT=ones_colb,
                             rhs=l0[:, ch * 512:(ch + 1) * 512],
                             start=True, stop=False)
            nc.tensor.matmul(cs_ps, lhsT=ones_colb,
                             rhs=l1[:, ch * 512:(ch + 1) * 512],
                             start=False, stop=True)
            nc.scalar.copy(out=cs_row[:, ch * 512:(ch + 1) * 512], in_=cs_ps)
        nc.sync.dma_start(out=ar_in[ARA:ARA + N], in_=cs_row)

        # ---- normalized old memory, transposed (bf16) ----
        mem3 = mem_sb.rearrange("p (t d) -> p t d", d=128)
        msq = wk.tile([128, NT], dt, tag="msq")
        for t in range(NT):
            sqd = wk.tile([128, 128], BF16, tag="sqd", bufs=2)
            nc.scalar.activation(out=sqd, in_=mem3[:, t, :], func=AF.Square,
                                 accum_out=msq[:, t:t + 1])
        nc.scalar.activation(out=msq, in_=msq, func=AF.Sqrt)
        nc.vector.tensor_scalar(out=msq, in0=msq, scalar1=1e-12, scalar2=None,
                                op0=ALU.max)
        nc.vector.reciprocal(out=msq, in_=msq)
        memnT = wk.tile([128, N], BF16, tag="mT")
        for t in range(NT):
            nm = wk.tile([128, 128], BF16, tag="nm", bufs=2)
            nc.scalar.activation(out=nm, in_=mem3[:, t, :], func=AF.Copy,
                                 scale=msq[:, t:t + 1])
            ps_tr = ppt.tile([128, 128], BF16, tag="tr")
            nc.tensor.transpose(ps_tr, nm, identb)
            nc.vector.tensor_copy(out=memnT[:, t * 128:(t + 1) * 128],
                                  in_=ps_tr)

        # ============ Stage B: LSTM (batch-stationary) ============
        # pass 1: gates i,g interleaved in a (32,2048) psum; pass 2: o.
        def lhs_k(k):
            if k < KH:
                return cin[:, k, :]
            if k < KC:
                return cmean
            return cin13

        ps_ig = ppq.tile([BL, 2048], dt, tag="quad")
        for k in range(KC + 1):
            lk = lhs_k(k)
            rhs = w3_sb[:, k * 3072:k * 3072 + 2048] if k < KC else w3b[:, 0:2048]
            for c in range(4):
                nc.tensor.matmul(ps_ig[:, c * 512:(c + 1) * 512], lhsT=lk,
                                 rhs=rhs[:, c * 512:(c + 1) * 512],
                                 start=(k == 0), stop=(k == KC))
        si = wk.tile([BL, 1024], dt, tag="l0")
        nc.scalar.activation(out=si, in_=ps_ig[:, 0:1024], func=AF.Sigmoid)
        tg = wk.tile([BL, 1024], dt, tag="l1")
        nc.scalar.activation(out=tg, in_=ps_ig[:, 1024:2048], func=AF.Tanh)
        ps_o = ppq.tile([BL, 1024], dt, tag="quad")
        for k in range(KC + 1):
            lk = lhs_k(k)
            rhs = (w3_sb[:, k * 3072 + 2048:(k + 1) * 3072] if k < KC
                   else w3b[:, 2048:3072])
            for c in range(2):
                nc.tensor.matmul(ps_o[:, c * 512:(c + 1) * 512], lhsT=lk,
                                 rhs=rhs[:, c * 512:(c + 1) * 512],
                                 start=(k == 0), stop=(k == KC))
        nc.vector.tensor_mul(out=si, in0=si, in1=tg)
        nc.scalar.activation(out=si, in_=si, func=AF.Tanh)
        nc.scalar.activation(out=tg, in_=ps_o, func=AF.Sigmoid)
        h_bf = wk.tile([BL, 1024], BF16, tag="hbf")
        nc.vector.tensor_mul(out=h_bf, in0=si, in1=tg)
        hT = wk.tile([128, KH, BL], BF16)
        for k in range(KH):
            ps_h = ppt.tile([128, BL], BF16, tag="tr")
            nc.tensor.transpose(ps_h, h_bf[:, k * 128:(k + 1) * 128],
                                identb[0:BL, 0:BL])
            nc.vector.tensor_copy(out=hT[:, k, :], in_=ps_h)

        # ============ Stage C: interface vector ============
        ps_itf = ppq.tile([BL, IF], dt, tag="quad")
        for k in range(KH):
            nc.tensor.matmul(ps_itf[:, 0:512], lhsT=hT[:, k, :],
                             rhs=wif_sb[:, k * IF:k * IF + 512],
                             start=(k == 0), stop=(k == KH - 1))
            nc.tensor.matmul(ps_itf[:, 512:IF], lhsT=hT[:, k, :],
                             rhs=wif_sb[:, k * IF + 512:(k + 1) * IF],
                             start=(k == 0), stop=(k == KH - 1))
        itf = wk.tile([BL, IF], dt, tag="itfb")
        nc.vector.tensor_tensor(out=itf, in0=ps_itf, in1=bif_b, op=ALU.add)

        wv = itf[:, 0:128]
        ersig = wk.tile([BL, 128], dt)
        nc.scalar.activation(out=ersig, in_=itf[:, 128:256], func=AF.Sigmoid)
        wgag = wk.tile([BL, 2], dt)
        nc.scalar.activation(out=wgag, in_=itf[:, 256:258], func=AF.Sigmoid)
        wg = wgag[:, 0:1]
        agt = wgag[:, 1:2]
        expm = wk.tile([BL, 12], dt)
        nc.scalar.activation(out=expm, in_=itf[:, 259:271], func=AF.Exp)
        msum = wk.tile([BL, 4], dt)
        nc.vector.tensor_reduce(out=msum,
                                in_=expm.rearrange("p (r k) -> p r k", k=3),
                                axis=mybir.AxisListType.X, op=ALU.add)
        minv = wk.tile([BL, 4], dt)
        nc.vector.reciprocal(out=minv, in_=msum)
        sc16 = wk.tile([BL, 16], dt)   # [rstr | m0 | m1 | m2]
        nc.scalar.activation(out=sc16[:, 0:4], in_=itf[:, 271:275],
                             func=AF.Exp)
        nc.scalar.activation(out=sc16[:, 0:4], in_=sc16[:, 0:4],
                             func=AF.Ln, bias=1.0)
        em3 = expm.rearrange("p (r k) -> p r k", k=3)
        for kk in range(3):
            nc.vector.tensor_mul(out=sc16[:, 4 + 4 * kk:8 + 4 * kk],
                                 in0=em3[:, :, kk], in1=minv)
        ps_t16 = ppt.tile([16, 32], dt, tag="tr")
        nc.tensor.transpose(ps_t16, sc16, ident[0:32, 0:32])
        t16 = wk.tile([16, 32], dt)
        nc.vector.tensor_copy(out=t16, in_=ps_t16)
        cols4 = wk.tile([128, 4], dt)  # [str | m0 | m1 | m2] as rb-columns
        for q in range(4):
            nc.sync.dma_start(out=cols4[:, q:q + 1],
                              in_=t16[4 * q:4 * q + 4, :])
        str_col = cols4[:, 0:1]
        m0_col = cols4[:, 1:2]

        ev_bf = wk.tile([BL, 128], BF16)
        nc.vector.tensor_scalar(out=ev_bf, in0=ersig, scalar1=wg, scalar2=None,
                                op0=ALU.mult)
        av_bf = wk.tile([BL, 128], BF16)
        nc.vector.tensor_scalar(out=av_bf, in0=wv, scalar1=wg, scalar2=None,
                                op0=ALU.mult)

        sq = wk.tile([BL, 128], dt, tag="sq")
        nrm = wk.tile([BL, 1], dt, tag="nrm")
        nc.scalar.activation(out=sq, in_=wv, func=AF.Square, accum_out=nrm)
        nc.scalar.activation(out=nrm, in_=nrm, func=AF.Sqrt)
        nc.vector.tensor_scalar(out=nrm, in0=nrm, scalar1=1e-12, scalar2=None,
                                op0=ALU.max)
        nc.vector.reciprocal(out=nrm, in_=nrm)
        nwv = wk.tile([BL, 128], BF16)
        nc.vector.tensor_scalar(out=nwv, in0=wv, scalar1=nrm, scalar2=None,
                                op0=ALU.mult)
        ps_nwvT = ppt.tile([128, BL], BF16, tag="tr")
        nc.tensor.transpose(ps_nwvT, nwv, identb[0:BL, 0:BL])
        nwvT = wk.tile([128, BL], BF16)
        nc.vector.tensor_copy(out=nwvT, in_=ps_nwvT)

        # normalized read keys, transposed (for post-collective sim)
        nkT = wk.tile([128, 128], BF16)
        rk3 = itf[:, 275:787].rearrange("p (r d) -> p r d", d=128)
        sqk3 = wk.tile([BL, R, 128], dt)
        nc.vector.tensor_mul(out=sqk3, in0=rk3, in1=rk3)
        nrk4 = wk.tile([BL, R], dt)
        nc.vector.tensor_reduce(out=nrk4, in_=sqk3, axis=mybir.AxisListType.X,
                                op=ALU.add)
        nc.scalar.activation(out=nrk4, in_=nrk4, func=AF.Sqrt)
        nc.vector.tensor_scalar(out=nrk4, in0=nrk4, scalar1=1e-12,
                                scalar2=None, op0=ALU.max)
        nc.vector.reciprocal(out=nrk4, in_=nrk4)
        nkb = wk.tile([BL, R, 128], BF16)
        nc.vector.tensor_tensor(out=nkb, in0=rk3,
                                in1=nrk4.unsqueeze(2).broadcast_to([BL, R, 128]),
                                op=ALU.mult)
        for r in range(R):
            ps_k = ppt.tile([128, BL], BF16, tag="tr")
            nc.tensor.transpose(ps_k, nkb[:, r, :], identb[0:BL, 0:BL])
            nc.vector.tensor_copy(out=nkT[:, r * BL:(r + 1) * BL], in_=ps_k)

        # m1/m2 columns transposed+cast for the post-collective term matmul
        ps_mt = ppt.tile([2, 128], dt, tag="tr")
        nc.tensor.transpose(ps_mt, cols4[:, 2:4], ident)
        m12T = wk.tile([2, 128], BF16)
        nc.vector.tensor_copy(out=m12T, in_=ps_mt)

        # ============ Stage D: write addressing + partials -> AllReduce =====
        ps_cw = ppq.tile([BL, N], dt, tag="quad")
        for ch in range(4):
            nc.tensor.matmul(ps_cw[:, ch * 512:(ch + 1) * 512], lhsT=nwvT,
                             rhs=memnT[:, ch * 512:(ch + 1) * 512],
                             start=True, stop=True)
        cwe_bf = wk.tile([BL, N], BF16, tag="cwe")
        den = wk.tile([BL, 1], dt, tag="den")
        nc.scalar.activation(out=cwe_bf, in_=ps_cw, func=AF.Exp, accum_out=den)
        nc.vector.reciprocal(out=den, in_=den)
        a_sc = wk.tile([BL, 1], dt)
        nc.vector.tensor_mul(out=a_sc, in0=wg, in1=den)
        nc.vector.tensor_scalar(out=a_sc, in0=a_sc, scalar1=0.5, scalar2=None,
                                op0=ALU.mult)
        b_sc = wk.tile([BL, 1], dt)
        nc.vector.tensor_mul(out=b_sc, in0=wg, in1=agt)
        nc.vector.tensor_scalar(out=b_sc, in0=b_sc, scalar1=0.5, scalar2=None,
                                op0=ALU.mult)
        ps_bt = ppt.tile([1, BL], dt, tag="tr")
        nc.tensor.transpose(ps_bt, b_sc, ident[0:BL, 0:BL])
        b_scT = wk.tile([1, BL], BF16)
        nc.vector.tensor_copy(out=b_scT, in_=ps_bt)

        alloc_r = wk.tile([1, N], dt, tag="allocr")
        nc.sync.dma_start(out=alloc_r, in_=ag_out[:, 0:NSH])
        al_bf = wk.tile([1, N], BF16, tag="albf")
        nc.scalar.copy(out=al_bf, in_=alloc_r)

        ps_w = ppq.tile([BL, N], dt, tag="quad")
        for ch in range(4):
            nc.tensor.matmul(ps_w[:, ch * 512:(ch + 1) * 512], lhsT=b_scT,
                             rhs=al_bf[:, ch * 512:(ch + 1) * 512],
                             start=True, stop=True)
        wt_bf = wk.tile([BL, N], BF16, tag="wt")
        for ch in range(4):
            nc.vector.scalar_tensor_tensor(
                out=wt_bf[:, ch * 512:(ch + 1) * 512],
                in0=cwe_bf[:, ch * 512:(ch + 1) * 512], scalar=a_sc,
                in1=ps_w[:, ch * 512:(ch + 1) * 512], op0=ALU.mult, op1=ALU.add)
        wsq_bf = wk.tile([BL, N], BF16, tag="wsq")
        nc.vector.tensor_mul(out=wsq_bf, in0=wt_bf, in1=wt_bf)
        s_col = wk.tile([BL, 1], dt, tag="scol")
        nc.vector.tensor_reduce(out=s_col, in_=wt_bf, axis=mybir.AxisListType.X,
                                op=ALU.add)
        rhs_eva = wk.tile([BL, 257], BF16)
        nc.vector.tensor_copy(out=rhs_eva[:, 0:128], in_=ev_bf)
        nc.vector.tensor_copy(out=rhs_eva[:, 128:256], in_=av_bf)
        nc.scalar.copy(out=rhs_eva[:, 256:257], in_=s_col)

        stag = wk.tile([128, NT, 256], BF16, tag="eaf", bufs=1)
        lustag = wk.tile([128, NT], BF16)
        for t in range(NT):
            ps_p = ppt.tile([128, 257], dt, tag="tr")
            nc.tensor.matmul(ps_p, lhsT=wt_bf[:, t * 128:(t + 1) * 128],
                             rhs=rhs_eva, start=True, stop=True)
            ps_q = ppt.tile([128, 1], dt, tag="tr")
            nc.tensor.matmul(ps_q, lhsT=wsq_bf[:, t * 128:(t + 1) * 128],
                             rhs=ones32b, start=True, stop=True)
            if t % 2 == 0:
                nc.vector.tensor_copy(out=stag[:, t, :], in_=ps_p[:, 0:256])
            else:
                nc.scalar.copy(out=stag[:, t, :], in_=ps_p[:, 0:256])
            qsb = wk.tile([128, 1], dt, tag="qsb", bufs=2)
            nc.scalar.copy(out=qsb, in_=ps_q)
            nc.vector.tensor_sub(out=lustag[:, t:t + 1], in0=ps_p[:, 256:257],
                                 in1=qsb)
        arA_in = ar_in[0:ARA].rearrange("(p t f) -> p t f", t=NT, f=256)
        for q in range(4):
            nc.sync.dma_start(out=arA_in[:, q * 4:(q + 1) * 4, :],
                              in_=stag[:, q * 4:(q + 1) * 4, :])
        nc.sync.dma_start(
            out=ar_in[ARA + N:ARA + 2 * N].rearrange("(t p) -> p t", p=128),
            in_=lustag)

        nc.gpsimd.collective_compute(
            "AllReduce", ALU.add, replica_groups=[list(range(M))],
            ins=[ar_in[:]], outs=[ar_out[:]])

        # ============ Stage F part 1: h @ Wout_h during the AllReduce =======
        psF = ppf.tile([BL, 1024], dt, tag="f")
        for k in list(range(KH)) + [KC]:
            lk = hT[:, k, :] if k < KH else cin13
            rhs = wout_sb[:, k * 1024:(k + 1) * 1024] if k < KC else woutb
            for c in range(2):
                nc.tensor.matmul(psF[:, c * 512:(c + 1) * 512], lhsT=lk,
                                 rhs=rhs[:, c * 512:(c + 1) * 512],
                                 start=(k == 0), stop=False,
                                 skip_group_check=True)

        # ============ Stage E: memory update + read weights ============
        ea = wk.tile([128, NT, 256], BF16, tag="wif", bufs=1)
        arA_out = ar_out[0:ARA].rearrange("(p t f) -> p t f", t=NT, f=256)
        for q in range(4):
            nc.sync.dma_start(out=ea[:, q * 4:(q + 1) * 4, :],
                              in_=arA_out[:, q * 4:(q + 1) * 4, :])
        rsg = wk.tile([1, N], dt, tag="ubig")
        nc.sync.dma_start(out=rsg, in_=ag_out[:, NSH:2 * NSH])
        rsb = wk.tile([1, N], BF16, tag="csrow")
        nc.scalar.copy(out=rsb, in_=rsg)
        cr2 = wk.tile([2, N], BF16, tag="l0")
        nc.sync.dma_start(out=cr2[0:1, :], in_=ar_out[ARA:ARA + N])
        nc.sync.dma_start(out=cr2[1:2, :], in_=rsb)
        lu2 = wk.tile([2, N], BF16, tag="l1")
        nc.sync.dma_start(out=lu2[0:1, :], in_=ar_out[ARA + N:ARA + 2 * N])
        nc.sync.dma_start(out=lu2[1:2, :], in_=ar_out[ARA + N:ARA + 2 * N])
        # BW/FW rows (bf16): 0.9/N * (col|row)sum + 0.1/(N*B) * lustat
        bwfw = wk.tile([2, N], BF16, tag="itfb")
        nc.vector.tensor_scalar(out=bwfw, in0=cr2, scalar1=0.9 / N,
                                scalar2=None, op0=ALU.mult)
        nc.vector.scalar_tensor_tensor(out=bwfw, in0=lu2,
                                       scalar=0.1 / (N * B), in1=bwfw,
                                       op0=ALU.mult, op1=ALU.add)

        # mnew (bf16): mem*(1-ep/B) + ap/B
        t1 = wk.tile([128, NT, 128], BF16, tag="ubig", bufs=1)
        nc.vector.tensor_tensor(out=t1, in0=mem3, in1=ea[:, :, 0:128],
                                op=ALU.mult)
        m1 = wk.tile([128, NT, 128], BF16, tag="lbig", bufs=1)
        nc.vector.scalar_tensor_tensor(out=m1, in0=ea[:, :, 128:256],
                                       scalar=1.0 / B, in1=mem3,
                                       op0=ALU.mult, op1=ALU.add)
        mn_bf = wk.tile([128, N], BF16, tag="mnbf")
        mnb3 = mn_bf.rearrange("p (t d) -> p t d", d=128)
        nc.vector.scalar_tensor_tensor(out=mnb3, in0=t1, scalar=-1.0 / B,
                                       in1=m1, op0=ALU.mult, op1=ALU.add)
        # row norms + normalized transpose
        msq2 = wk.tile([128, NT], dt, tag="msq2")
        for t in range(NT):
            sqd = wk.tile([128, 128], BF16, tag="sqd", bufs=2)
            if t % 2 == 0:
                nc.scalar.activation(out=sqd, in_=mnb3[:, t, :],
                                     func=AF.Square,
                                     accum_out=msq2[:, t:t + 1])
            else:
                nc.vector.tensor_mul(out=sqd, in0=mnb3[:, t, :],
                                     in1=mnb3[:, t, :])
                nc.vector.tensor_reduce(out=msq2[:, t:t + 1], in_=sqd,
                                        axis=mybir.AxisListType.X, op=ALU.add)
        nc.scalar.activation(out=msq2, in_=msq2, func=AF.Sqrt)
        nc.vector.tensor_scalar(out=msq2, in0=msq2, scalar1=1e-12,
                                scalar2=None, op0=ALU.max)
        nc.vector.reciprocal(out=msq2, in_=msq2)
        mnewT = wk.tile([128, N], BF16, tag="mT")
        for t in range(NT):
            nm = wk.tile([128, 128], BF16, tag="nm", bufs=2)
            if t % 2 == 0:
                nc.scalar.activation(out=nm, in_=mnb3[:, t, :], func=AF.Copy,
                                     scale=msq2[:, t:t + 1])
            else:
                nc.vector.tensor_scalar(out=nm, in0=mnb3[:, t, :],
                                        scalar1=msq2[:, t:t + 1],
                                        scalar2=None, op0=ALU.mult)
            ps_tr = ppt.tile([128, 128], BF16, tag="tr")
            nc.tensor.transpose(ps_tr, nm, identb)
            nc.vector.tensor_copy(out=mnewT[:, t * 128:(t + 1) * 128],
                                  in_=ps_tr)

        ps_sim = ppq.tile([128, N], dt, tag="quad")
        for ch in range(4):
            nc.tensor.matmul(ps_sim[:, ch * 512:(ch + 1) * 512], lhsT=nkT,
                             rhs=mnewT[:, ch * 512:(ch + 1) * 512],
                             start=True, stop=True)
        esim = wk.tile([128, N], BF16, tag="cwe")
        dsum = wk.tile([128, 1], dt)
        nc.scalar.activation(out=esim, in_=ps_sim, func=AF.Exp, scale=str_col,
                             accum_out=dsum)
        nc.vector.reciprocal(out=dsum, in_=dsum)
        c0 = wk.tile([128, 1], dt)
        nc.vector.tensor_mul(out=c0, in0=m0_col, in1=dsum)
        ps_term = ppq.tile([128, N], dt, tag="quad")
        for ch in range(4):
            nc.tensor.matmul(ps_term[:, ch * 512:(ch + 1) * 512], lhsT=m12T,
                             rhs=bwfw[:, ch * 512:(ch + 1) * 512],
                             start=True, stop=True)
        nrw = wk.tile([128, N], BF16, tag="wt")
        for ch in range(4):
            nc.vector.scalar_tensor_tensor(
                out=nrw[:, ch * 512:(ch + 1) * 512],
                in0=esim[:, ch * 512:(ch + 1) * 512], scalar=c0,
                in1=ps_term[:, ch * 512:(ch + 1) * 512], op0=ALU.mult,
                op1=ALU.add)

        ps_ro = ppq.tile([128, 128], dt, tag="quad")
        for t in range(NT):
            ps_tr = ppt.tile([128, 128], BF16, tag="tr")
            nc.tensor.transpose(ps_tr, nrw[:, t * 128:(t + 1) * 128], identb)
            nrwT = wk.tile([128, 128], BF16, tag="nrwT", bufs=2)
            nc.vector.tensor_copy(out=nrwT, in_=ps_tr)
            nc.tensor.matmul(ps_ro, lhsT=mn_bf[:, t * 128:(t + 1) * 128],
                             rhs=nrwT, start=(t == 0), stop=(t == NT - 1))
        roT = wk.tile([128, 128], BF16)
        nc.scalar.copy(out=roT, in_=ps_ro)

        # ============ Stage F part 2: + ro @ Wout_ro, write out ============
        for k in range(KH, KC):
            lk = roT[:, (k - KH) * BL:(k - KH + 1) * BL]
            for c in range(2):
                nc.tensor.matmul(psF[:, c * 512:(c + 1) * 512], lhsT=lk,
                                 rhs=wout_sb[:, k * 1024 + c * 512:
                                             k * 1024 + (c + 1) * 512],
                                 start=False, stop=(k == KC - 1 and c == 1),
                                 skip_group_check=True)
        out_sb = wk.tile([BL, 1024], dt, tag="wsq")
        nc.scalar.copy(out=out_sb[:, 0:512], in_=psF[:, 0:512])
        nc.vector.tensor_copy(out=out_sb[:, 512:1024], in_=psF[:, 512:1024])
        nc.sync.dma_start(out=outO[:, 0:512], in_=out_sb[:, 0:512])
        nc.sync.dma_start(out=outO[:, 512:1024], in_=out_sb[:, 512:1024])

    nc.finalize()
    return nc


def _prep_inputs(x, memory, usage, link, W_ih, W_hh, b_ih, b_hh, W_if, b_if,
                 W_out, b_out):
    f = np.float32
    bf = ml_dtypes.bfloat16
    x = np.asarray(x, f); memory = np.asarray(memory, f)
    usage = np.asarray(usage, f); link = np.asarray(link, f)
    W_ih = np.asarray(W_ih, f); b_ih = np.asarray(b_ih, f)
    b_hh = np.asarray(b_hh, f); W_if = np.asarray(W_if, f)
    b_if = np.asarray(b_if, f); W_out = np.asarray(W_out, f)
    b_out = np.asarray(b_out, f)

    sel = np.r_[0:1024, 2048:4096]
    W3T = W_ih[sel].T                             # (1536, 3072) cols [i|g|o]
    w3 = np.ascontiguousarray(
        W3T.reshape(KC, 128, 3072).transpose(1, 0, 2)
        .reshape(128, KC * 3072).astype(bf))
    b3r = ((b_ih + b_hh)[sel]).reshape(1, 3072).astype(bf)
    wif = np.ascontiguousarray(
        W_if.T.reshape(KH, 128, IF).transpose(1, 0, 2)
        .reshape(128, KH * IF).astype(bf))
    wout = np.ascontiguousarray(
        W_out.T.reshape(KC, 128, 1024).transpose(1, 0, 2)
        .reshape(128, KC * 1024).astype(bf))
    borr = b_out.reshape(1, 1024).astype(bf)
    memA = np.ascontiguousarray(
        memory.reshape(NT, 128, 128).transpose(1, 0, 2).reshape(128, N)
        .astype(bf))
    bifr = b_if.reshape(1, IF).astype(bf)
    usg = usage.reshape(1, N)

    shared = dict(w3=w3, b3r=b3r, wif=wif, bifr=bifr, wout=wout, borr=borr,
                  memA=memA, usg=usg)
    in_maps = []
    for c in range(M):
        xs = x[c * BL:(c + 1) * BL]               # (32, 1024)
        cinx = np.ascontiguousarray(
            xs.T.reshape(KH, 128, BL).transpose(1, 0, 2)
            .reshape(128, KH * BL).astype(bf))
        ls = link[c * 256:(c + 1) * 256]          # (256, 2048)
        lnkm = np.ascontiguousarray(
            ls.reshape(2, 128, N).transpose(1, 0, 2).reshape(128, 2 * N)
            .astype(bf))
        ucols = np.ascontiguousarray(
            usage.reshape(NT, 128)[2 * c:2 * c + 2].T)      # (128, 2)
        m = dict(shared)
        m["cinx"] = cinx
        m["lnk"] = lnkm
        m["ucols"] = ucols
        in_maps.append(m)
    return in_maps


def kernel(**inputs):
    nc = build_nc()
    in_maps = _prep_inputs(**inputs)
    res = run_bass_kernel_spmd(nc, in_maps, list(range(M))).results
    outs = [res[c]["outO"] for c in range(M)]     # (32, 1024) each
    return np.concatenate(outs, 0).astype(np.float32)
